# revision 50
# baseline (speedup 1.0000x reference)
"""Trainium2 Bass kernel for LpAlignEntropyLoss (B=2048, D=128, 2 views).

loss = mean_i ||z0_i - z1_i + eps||  -  0.5 * sum_v mean_i [ logsumexp_{j!=i}(-||zv_i - zv_j + eps||) - log(B-1) ]

Symmetric block scheme (8 NeuronCores, 256 rows/core):
  The BxB distance matrix is symmetric, so core c only computes blocks
  (c, c..c+4): gathered columns are the 1280 cyclically-next rows. Row
  sums come from the fused ACT accum; the mirrored contributions for
  blocks k=1..3 are column sums of the exp tiles, which are DMA'd out
  and reduced on the host. Block k=4 is computed by both endpoints
  (row-sums only), keeping the SPMD program uniform.

  dist^2[i,j] = n_i + n_j - 2 z_i.z_j, assembled fully in PSUM:
  - PE: psum = (-2 z_i).z_j (bf16 lhsT, host-prescaled) + [1;n_i]x[n_j;1]
    (K=2 aug matmul) + BIG*I (identity matmul, masks the diagonal).
  - DVE: sqrt via the fp32 bit trick -- psum bitcast to int32,
    dist_bits = 0.5*i + MAGIC (one tensor_scalar mult+add).  MAGIC is
    tuned so the logsumexp bias cancels (validated ~3e-7 rel).
  - ACT: Exp(-dist) on dist bitcast to f32, fused accum_out row-sum.
    Only the exp table is ever loaded (preloaded at t=0 via a dummy).
  Host finishes the tail: align term, mirror column sums, log, means.
"""
import numpy as np
import ml_dtypes
from contextlib import ExitStack

B = 2048
D = 128
N_CORES = 8
R = B // N_CORES          # 256 rows per core
G = 1280                  # gathered columns per core (5 blocks of 256)
MAGIC = 532626640.0       # sqrt bit-trick offset, tuned on the data model
CEXP = 16.5               # exp bias: dumps hold exp(CEXP - d) in fp8 e4m3
BIG = float(2 ** 20)
LOG_NM1 = float(np.log(B - 1))

# dump slabs DMA'd out; host derives both row sums and mirror column sums:
#   name -> (view, chunk, piece_c0, slice_lo, slice_hi)  (gathered cols)
DUMPS = [
    ("d00a", 0, 0, 0, 0, 512),
    ("d00b", 0, 0, 512, 512, 1280),
    ("d01", 0, 1, 0, 0, 1280),
    ("d10", 1, 0, 0, 0, 1280),
    ("d11a", 1, 1, 256, 256, 1024),
]
assert len(DUMPS) == 5

_cache: dict = {}


def _build():
    import concourse.tile as tile
    from concourse import bacc, mybir

    f32 = mybir.dt.float32
    bf16 = mybir.dt.bfloat16
    f8 = mybir.dt.float8e4
    i32 = mybir.dt.int32
    AF = mybir.ActivationFunctionType
    ALU = mybir.AluOpType

    nc = bacc.Bacc("TRN2", target_bir_lowering=False, debug=False,
                   num_devices=N_CORES)

    zc0b_d = nc.dram_tensor("zc0b", [D, G - 512], bf16, kind="ExternalInput").ap()
    zc1_d = nc.dram_tensor("zc1", [D, G], bf16, kind="ExternalInput").ap()
    # blob0 cols: zl0(256) ident(128) ibig(128); blob1: zl1(256)
    blob0_d = nc.dram_tensor("blob0", [128, 1024], bf16, kind="ExternalInput").ap()
    blob1_d = nc.dram_tensor("blob1", [128, 256], bf16, kind="ExternalInput").ap()
    # aug cols per view: [2,256] lhsT region (1; n_own), [2,G] rhs (n_gath; 1)
    aug_d = nc.dram_tensor("aug", [2, 2 * (256 + G)], bf16,
                           kind="ExternalInput").ap()
    accs_d = nc.dram_tensor("accs", [128, 2], f32, kind="ExternalOutput").ap()
    dump_d = {name: nc.dram_tensor(name, [128, hi - lo], f8,
                                   kind="ExternalOutput").ap()
              for name, v, t, c0, lo, hi in DUMPS}

    with tile.TileContext(nc) as tc, ExitStack() as ctx:
        consts = ctx.enter_context(tc.tile_pool(name="consts", bufs=1))
        psum = ctx.enter_context(tc.tile_pool(name="psum", bufs=4, space="PSUM"))
        distp = ctx.enter_context(tc.tile_pool(name="distp", bufs=3))
        dumpp = ctx.enter_context(tc.tile_pool(name="dumpp", bufs=6))

        # --- ACT exp-table preload (no data deps) ---
        dumm = consts.tile([128, 1], f32, tag="dumm")
        nc.vector.memset(dumm[:], 0.0)
        cexp = consts.tile([128, 1], f32, tag="cexp")
        nc.vector.memset(cexp[:], CEXP)
        dumo = consts.tile([128, 1], bf16, tag="dumo")
        nc.scalar.activation(dumo[:], dumm[:], AF.Exp, scale=-1.0)

        # --- PE p-state warm-up: small matmuls from t~0.3us keep PE busy
        # until real work arrives, so real matmuls run at 2.4GHz ---
        dumw = consts.tile([128, 128], bf16, tag="dumw")
        nc.gpsimd.memset(dumw[:], 0.0)
        dumP = psum.tile([128, 1024], f32, tag="P")
        for _ in range(26):
            nc.tensor.matmul(dumP[:, 0:128], dumw[:], dumw[:],
                             start=True, stop=True)

        # --- inputs ---
        sb_zc0 = consts.tile([D, G], bf16, tag="zc0")
        sb_zc1 = consts.tile([D, G], bf16, tag="zc1")
        sb_zc = [sb_zc0, sb_zc1]
        blob0 = consts.tile([128, 1024], bf16, tag="blob0")
        blob1 = consts.tile([128, 256], bf16, tag="blob1")
        aug = consts.tile([2, 2 * (256 + G)], bf16, tag="aug")
        nc.sync.dma_start(blob0[:], blob0_d)
        nc.sync.dma_start(aug[:], aug_d)
        nc.sync.dma_start(sb_zc[0][:, 512:G], zc0b_d)
        nc.sync.dma_start(sb_zc[1][:], zc1_d)
        nc.sync.dma_start(blob1[:], blob1_d)

        zl = [blob0[:, 0:256], blob1[:, 0:256]]
        ident = blob0[:, 256:384]
        ibig = blob0[:, 384:512]
        augl = [aug[:, 0:256], aug[:, 256 + G:512 + G]]
        augr = [aug[:, 256:256 + G], aug[:, 512 + G:512 + 2 * G]]

        accs = consts.tile([128, 2], f32, tag="accs")
        dists = {}
        dumps = {}

        def mm_trick(v, t, c0, c1, dist, act_trick=False):
            """matmuls + sqrt-trick for gathered cols [c0,c1) into dist."""
            ncols = c1 - c0
            P = psum.tile([128, ncols], f32, tag="P")
            lhsT = zl[v][:, t * 128:(t + 1) * 128]
            auglT = augl[v][:, t * 128:(t + 1) * 128]
            dlo, dhi = t * 128, (t + 1) * 128
            has_diag = c0 <= dlo and dhi <= c1
            for s0 in range(c0, c1, 512):
                s1 = min(s0 + 512, c1)
                sl = slice(s0 - c0, s1 - c0)
                rhs = (blob0[:, 512 + s0:512 + s1] if v == 0 and s1 <= 512
                       else sb_zc[v][:, s0:s1])
                nc.tensor.matmul(P[:, sl], lhsT, rhs,
                                 start=True, stop=False)
                last = not (has_diag and s0 <= dlo < s1)
                nc.tensor.matmul(P[:, sl], auglT, augr[v][:, s0:s1],
                                 start=False, stop=last)
            if has_diag:
                nc.tensor.matmul(P[:, dlo - c0:dhi - c0], ident, ibig,
                                 start=False, stop=True)
            if act_trick:
                # ACT is idle before its exp chain: same 0.5*i + MAGIC math
                # via Copy (float bias allowed), int32-convert on write
                nc.scalar.activation(dist[:, c0:c1].bitcast(i32),
                                     P[:].bitcast(i32), AF.Copy,
                                     bias=MAGIC, scale=0.5)
            else:
                nc.vector.tensor_scalar(dist[:, c0:c1].bitcast(i32),
                                        P[:].bitcast(i32), 0.5, MAGIC,
                                        ALU.mult, ALU.add)

        def exp_acc(v, t, c0, c1, acc_idx=None):
            """exp over dist cols [c0,c1); host sums the dumped slab,
            except the last unit which uses the fused accum row-sum."""
            dmp = dumpp.tile([128, c1 - c0], f8, tag="dump")
            acc = None if acc_idx is None else accs[:, acc_idx:acc_idx + 1]
            nc.scalar.activation(dmp[:], dists[(v, t)][:, c0:c1], AF.Exp,
                                 scale=-1.0, bias=cexp[:], accum_out=acc)
            dumps[(v, t, c0)] = dmp

        def dump_out(name, v, t, c0, lo, hi):
            dmp = dumps[(v, t, c0)]
            nc.sync.dma_start(dump_d[name], dmp[:, lo - c0:hi - c0])

        for v, t in [(0, 0), (0, 1), (1, 0), (1, 1)]:
            dtile = distp.tile([128, G], f32, tag=f"dist{v}{t}")
            dists[(v, t)] = dtile

        # u00 in 2 pieces for an early ACT start; u11 split so its mirror
        # slab (cols 256..1024) ships before the final k4 piece finishes.
        mm_trick(0, 0, 0, 512, dists[(0, 0)])
        exp_acc(0, 0, 0, 512)
        mm_trick(0, 0, 512, 1024, dists[(0, 0)])
        mm_trick(0, 0, 1024, G, dists[(0, 0)])
        exp_acc(0, 0, 512, G)
        dump_out(*DUMPS[0])
        mm_trick(0, 1, 0, 1024, dists[(0, 1)])
        mm_trick(0, 1, 1024, G, dists[(0, 1)])
        exp_acc(0, 1, 0, G)
        dump_out(*DUMPS[1])
        mm_trick(1, 0, 0, 1024, dists[(1, 0)])
        mm_trick(1, 0, 1024, G, dists[(1, 0)])
        exp_acc(1, 0, 0, G)
        dump_out(*DUMPS[2])
        mm_trick(1, 1, 256, G, dists[(1, 1)])
        dump_out(*DUMPS[3])
        exp_acc(1, 1, 256, G, 0)
        dump_out(*DUMPS[4])
        mm_trick(1, 1, 0, 256, dists[(1, 1)])
        exp_acc(1, 1, 0, 256, 1)
        nc.sync.dma_start(accs_d, accs[:])

    nc.compile()
    return nc


def _prep_inputs(z0: np.ndarray, z1: np.ndarray):
    """Per-core input maps: gathered columns are the cyclically-next 1280."""
    bf = ml_dtypes.bfloat16
    zs = [np.ascontiguousarray(z0, np.float32), np.ascontiguousarray(z1, np.float32)]
    norms = [(z.astype(np.float64) ** 2).sum(-1) for z in zs]  # [B]
    eye = np.eye(128, dtype=np.float32)
    ident = eye.astype(bf)
    ibig = (BIG * eye).astype(bf)
    in_maps = []
    for c in range(N_CORES):
        gcols = (np.arange(G) + c * R) % B
        m = {}
        aug = np.zeros((2, 2 * (256 + G)), np.float32)
        zcs = []
        for v in (0, 1):
            zc = np.ascontiguousarray(zs[v][gcols].T).astype(bf)   # [D, G]
            zcs.append(zc)
            ng = norms[v][gcols].astype(np.float32)
            o = v * (256 + G)
            aug[0, o:o + 256] = 1.0
            aug[1, o:o + 256] = ng[:256]
            aug[0, o + 256:o + 256 + G] = ng
            aug[1, o + 256:o + 256 + G] = 1.0
        zl0 = (-2.0 * zcs[0][:, :256].astype(np.float32)).astype(bf)
        zl1 = (-2.0 * zcs[1][:, :256].astype(np.float32)).astype(bf)
        m["zc0b"] = np.ascontiguousarray(zcs[0][:, 512:])
        m["zc1"] = zcs[1]
        m["blob0"] = np.ascontiguousarray(
            np.concatenate([zl0, ident, ibig, zcs[0][:, :512]], axis=1))
        m["blob1"] = zl1
        m["aug"] = aug.astype(bf)
        in_maps.append(m)
    return in_maps


def kernel(z0: np.ndarray, z1: np.ndarray) -> np.ndarray:
    from concourse.bass_utils import run_bass_kernel_spmd

    if "nc" not in _cache:
        _cache["nc"] = _build()
    nc = _cache["nc"]

    in_maps = _prep_inputs(z0, z1)
    res = run_bass_kernel_spmd(nc, in_maps, core_ids=list(range(N_CORES)))

    rowsums = np.zeros((2, B), np.float64)   # [view, global row]
    for c in range(N_CORES):
        out = res.results[c]
        escale = float(np.exp(-CEXP))
        acc = out["accs"].astype(np.float64) * escale      # [128, 2]
        rowsums[1, c * R + 128:c * R + 256] += acc[:, 0] + acc[:, 1]
        for name, v, t, c0, lo, hi in DUMPS:
            slab = out[name].astype(np.float64) * escale    # [128, hi-lo]
            # own rows: sum along columns (u11's rows come from accums)
            if name != "d11a":
                r0 = c * R + t * 128
                rowsums[v, r0:r0 + 128] += slab.sum(axis=1)
            # mirrored rows (blocks k=1..3): sum along partitions
            mlo, mhi = max(lo, 256), min(hi, 1024)
            if mlo < mhi:
                cs = slab[:, mlo - lo:mhi - lo].sum(axis=0)
                grows = (np.arange(mlo, mhi) + c * R) % B
                rowsums[v, grows] += cs

    z0f = z0.astype(np.float64)
    z1f = z1.astype(np.float64)
    align_loss = np.sqrt(((z0f - z1f) ** 2).sum(-1)).mean()
    lme = np.log(rowsums) - LOG_NM1             # [2, B]
    entropy_loss = lme.mean()
    return np.float32(align_loss - entropy_loss)


# revision 51
# speedup vs baseline: 1.0483x; 1.0483x over previous
"""Trainium2 Bass kernel for LpAlignEntropyLoss (B=2048, D=128, 2 views).

loss = mean_i ||z0_i - z1_i + eps||  -  0.5 * sum_v mean_i [ logsumexp_{j!=i}(-||zv_i - zv_j + eps||) - log(B-1) ]

Symmetric block scheme (8 NeuronCores, 256 rows/core):
  The BxB distance matrix is symmetric, so core c only computes blocks
  (c, c..c+4): gathered columns are the 1280 cyclically-next rows. Row
  sums come from the fused ACT accum; the mirrored contributions for
  blocks k=1..3 are column sums of the exp tiles, which are DMA'd out
  and reduced on the host. Block k=4 is computed by both endpoints
  (row-sums only), keeping the SPMD program uniform.

  dist^2[i,j] = n_i + n_j - 2 z_i.z_j, assembled fully in PSUM:
  - PE: psum = (-2 z_i).z_j (bf16 lhsT, host-prescaled) + [1;n_i]x[n_j;1]
    (K=2 aug matmul) + BIG*I (identity matmul, masks the diagonal).
  - DVE: sqrt via the fp32 bit trick -- psum bitcast to int32,
    dist_bits = 0.5*i + MAGIC (one tensor_scalar mult+add).  MAGIC is
    tuned so the logsumexp bias cancels (validated ~3e-7 rel).
  - ACT: Exp(-dist) on dist bitcast to f32, fused accum_out row-sum.
    Only the exp table is ever loaded (preloaded at t=0 via a dummy).
  Host finishes the tail: align term, mirror column sums, log, means.
"""
import numpy as np
import ml_dtypes
from contextlib import ExitStack

B = 2048
D = 128
N_CORES = 8
R = B // N_CORES          # 256 rows per core
G = 1280                  # zc layout columns (k4 halves for both chunks)
GC = 1152                 # per-chunk compute span (k4 block halved)
MAGIC = 532626640.0       # sqrt bit-trick offset, tuned on the data model
CEXP = 16.5               # exp bias: dumps hold exp(CEXP - d) in fp8 e4m3
BIG = float(2 ** 20)
LOG_NM1 = float(np.log(B - 1))

# dump slabs DMA'd out; host derives both row sums and mirror column sums:
#   name -> (view, chunk, piece_c0, slice_lo, slice_hi)  (chunk-space cols)
DUMPS = [
    ("d00a", 0, 0, 0, 0, 512),
    ("d00b", 0, 0, 512, 512, GC),
    ("d01", 0, 1, 0, 0, GC),
    ("d10", 1, 0, 0, 0, GC),
    ("d11a", 1, 1, 256, 256, GC),
]

_cache: dict = {}


def _build():
    import concourse.tile as tile
    from concourse import bacc, mybir

    f32 = mybir.dt.float32
    bf16 = mybir.dt.bfloat16
    f8 = mybir.dt.float8e4
    i32 = mybir.dt.int32
    AF = mybir.ActivationFunctionType
    ALU = mybir.AluOpType

    nc = bacc.Bacc("TRN2", target_bir_lowering=False, debug=False,
                   num_devices=N_CORES)

    zc0b_d = nc.dram_tensor("zc0b", [D, G - 512], bf16, kind="ExternalInput").ap()
    zc1_d = nc.dram_tensor("zc1", [D, G], bf16, kind="ExternalInput").ap()
    # blob0 cols: zl0(256) ident(128) ibig(128); blob1: zl1(256)
    blob0_d = nc.dram_tensor("blob0", [128, 1024], bf16, kind="ExternalInput").ap()
    blob1_d = nc.dram_tensor("blob1", [128, 256], bf16, kind="ExternalInput").ap()
    # aug cols per view: [2,256] lhsT region (1; n_own), [2,G] rhs (n_gath; 1)
    aug_d = nc.dram_tensor("aug", [2, 2 * (256 + G)], bf16,
                           kind="ExternalInput").ap()
    accs_d = nc.dram_tensor("accs", [128, 2], f32, kind="ExternalOutput").ap()
    dump_d = {name: nc.dram_tensor(name, [128, hi - lo], f8,
                                   kind="ExternalOutput").ap()
              for name, v, t, c0, lo, hi in DUMPS}

    with tile.TileContext(nc) as tc, ExitStack() as ctx:
        consts = ctx.enter_context(tc.tile_pool(name="consts", bufs=1))
        psum = ctx.enter_context(tc.tile_pool(name="psum", bufs=4, space="PSUM"))
        distp = ctx.enter_context(tc.tile_pool(name="distp", bufs=3))
        dumpp = ctx.enter_context(tc.tile_pool(name="dumpp", bufs=6))

        # --- ACT exp-table preload (no data deps) ---
        dumm = consts.tile([128, 1], f32, tag="dumm")
        nc.vector.memset(dumm[:], 0.0)
        cexp = consts.tile([128, 1], f32, tag="cexp")
        nc.vector.memset(cexp[:], CEXP)
        dumo = consts.tile([128, 1], bf16, tag="dumo")
        nc.scalar.activation(dumo[:], dumm[:], AF.Exp, scale=-1.0)

        # --- PE p-state warm-up: small matmuls from t~0.3us keep PE busy
        # until real work arrives, so real matmuls run at 2.4GHz ---
        dumw = consts.tile([128, 128], bf16, tag="dumw")
        nc.gpsimd.memset(dumw[:], 0.0)
        dumP = psum.tile([128, 1024], f32, tag="P")
        for _ in range(26):
            nc.tensor.matmul(dumP[:, 0:128], dumw[:], dumw[:],
                             start=True, stop=True)

        # --- inputs ---
        sb_zc0 = consts.tile([D, G], bf16, tag="zc0")
        sb_zc1 = consts.tile([D, G], bf16, tag="zc1")
        sb_zc = [sb_zc0, sb_zc1]
        blob0 = consts.tile([128, 1024], bf16, tag="blob0")
        blob1 = consts.tile([128, 256], bf16, tag="blob1")
        aug = consts.tile([2, 2 * (256 + G)], bf16, tag="aug")
        nc.sync.dma_start(blob0[:], blob0_d)
        nc.sync.dma_start(aug[:], aug_d)
        nc.sync.dma_start(sb_zc[0][:, 512:G], zc0b_d)
        nc.sync.dma_start(sb_zc[1][:], zc1_d)
        nc.sync.dma_start(blob1[:], blob1_d)

        zl = [blob0[:, 0:256], blob1[:, 0:256]]
        ident = blob0[:, 256:384]
        ibig = blob0[:, 384:512]
        augl = [aug[:, 0:256], aug[:, 256 + G:512 + G]]
        augr = [aug[:, 256:256 + G], aug[:, 512 + G:512 + 2 * G]]

        accs = consts.tile([128, 2], f32, tag="accs")
        dists = {}
        dumps = {}

        def mm_trick(v, t, c0, c1, dist, act_trick=False):
            """matmuls + sqrt-trick for gathered cols [c0,c1) into dist."""
            ncols = c1 - c0
            P = psum.tile([128, ncols], f32, tag="P")
            lhsT = zl[v][:, t * 128:(t + 1) * 128]
            auglT = augl[v][:, t * 128:(t + 1) * 128]
            dlo, dhi = t * 128, (t + 1) * 128
            has_diag = c0 <= dlo and dhi <= c1
            cuts = list(range(c0, min(c1, 1024), 512)) + (
                [1024] if c1 > 1024 else [])
            for s0 in cuts:
                s1 = min(s0 + 512, c1) if s0 < 1024 else c1
                s1 = min(s1, 1024) if s0 < 1024 else s1
                sl = slice(s0 - c0, s1 - c0)
                # chunk-space [1024:1152) maps to zc cols [1024+128t, ...)
                z0c, z1c = (s0, s1) if s0 < 1024 else (s0 + 128 * t,
                                                       s1 + 128 * t)
                rhs = (blob0[:, 512 + z0c:512 + z1c] if v == 0 and z1c <= 512
                       else sb_zc[v][:, z0c:z1c])
                nc.tensor.matmul(P[:, sl], lhsT, rhs,
                                 start=True, stop=False)
                last = not (has_diag and s0 <= dlo < s1)
                nc.tensor.matmul(P[:, sl], auglT, augr[v][:, z0c:z1c],
                                 start=False, stop=last)
            if has_diag:
                nc.tensor.matmul(P[:, dlo - c0:dhi - c0], ident, ibig,
                                 start=False, stop=True)
            if act_trick:
                # ACT is idle before its exp chain: same 0.5*i + MAGIC math
                # via Copy (float bias allowed), int32-convert on write
                nc.scalar.activation(dist[:, c0:c1].bitcast(i32),
                                     P[:].bitcast(i32), AF.Copy,
                                     bias=MAGIC, scale=0.5)
            else:
                nc.vector.tensor_scalar(dist[:, c0:c1].bitcast(i32),
                                        P[:].bitcast(i32), 0.5, MAGIC,
                                        ALU.mult, ALU.add)

        def exp_acc(v, t, c0, c1, acc_idx=None):
            """exp over dist cols [c0,c1); host sums the dumped slab,
            except the last unit which uses the fused accum row-sum."""
            dmp = dumpp.tile([128, c1 - c0], f8, tag="dump")
            acc = None if acc_idx is None else accs[:, acc_idx:acc_idx + 1]
            nc.scalar.activation(dmp[:], dists[(v, t)][:, c0:c1], AF.Exp,
                                 scale=-1.0, bias=cexp[:], accum_out=acc)
            dumps[(v, t, c0)] = dmp

        def dump_out(name, v, t, c0, lo, hi):
            dmp = dumps[(v, t, c0)]
            nc.sync.dma_start(dump_d[name], dmp[:, lo - c0:hi - c0])

        for v, t in [(0, 0), (0, 1), (1, 0), (1, 1)]:
            dtile = distp.tile([128, GC], f32, tag=f"dist{v}{t}")
            dists[(v, t)] = dtile

        # u00 in 2 pieces for an early ACT start; u11 split so its mirror
        # slab (cols 256..1024) ships before the final k4 piece finishes.
        mm_trick(0, 0, 0, 512, dists[(0, 0)])
        exp_acc(0, 0, 0, 512)
        mm_trick(0, 0, 512, GC, dists[(0, 0)])
        exp_acc(0, 0, 512, GC)
        dump_out(*DUMPS[0])
        mm_trick(0, 1, 0, 768, dists[(0, 1)])
        mm_trick(0, 1, 768, GC, dists[(0, 1)])
        exp_acc(0, 1, 0, GC)
        dump_out(*DUMPS[1])
        mm_trick(1, 0, 0, 768, dists[(1, 0)])
        mm_trick(1, 0, 768, GC, dists[(1, 0)])
        exp_acc(1, 0, 0, GC)
        dump_out(*DUMPS[2])
        mm_trick(1, 1, 256, GC, dists[(1, 1)])
        dump_out(*DUMPS[3])
        exp_acc(1, 1, 256, GC, 0)
        dump_out(*DUMPS[4])
        mm_trick(1, 1, 0, 256, dists[(1, 1)])
        exp_acc(1, 1, 0, 256, 1)
        nc.sync.dma_start(accs_d, accs[:])

    nc.compile()
    return nc


def _prep_inputs(z0: np.ndarray, z1: np.ndarray):
    """Per-core input maps: gathered columns are the cyclically-next 1280."""
    bf = ml_dtypes.bfloat16
    zs = [np.ascontiguousarray(z0, np.float32), np.ascontiguousarray(z1, np.float32)]
    norms = [(z.astype(np.float64) ** 2).sum(-1) for z in zs]  # [B]
    eye = np.eye(128, dtype=np.float32)
    ident = eye.astype(bf)
    ibig = (BIG * eye).astype(bf)
    in_maps = []
    for c in range(N_CORES):
        base = (np.arange(1024) + c * R) % B
        if c < 4:
            k4t = [np.arange((c + 4) * R + 128 * t,
                             (c + 4) * R + 128 * t + 128) for t in (0, 1)]
        else:
            k4t = [np.arange((c - 4) * R + 128 * (1 - t),
                             (c - 4) * R + 128 * (1 - t) + 128) for t in (0, 1)]
        gcols = np.concatenate([base, k4t[0], k4t[1]]) % B
        m = {}
        aug = np.zeros((2, 2 * (256 + G)), np.float32)
        zcs = []
        for v in (0, 1):
            zc = np.ascontiguousarray(zs[v][gcols].T).astype(bf)   # [D, G]
            zcs.append(zc)
            ng = norms[v][gcols].astype(np.float32)
            o = v * (256 + G)
            aug[0, o:o + 256] = 1.0
            aug[1, o:o + 256] = ng[:256]
            aug[0, o + 256:o + 256 + G] = ng
            aug[1, o + 256:o + 256 + G] = 1.0
        zl0 = (-2.0 * zcs[0][:, :256].astype(np.float32)).astype(bf)
        zl1 = (-2.0 * zcs[1][:, :256].astype(np.float32)).astype(bf)
        m["zc0b"] = np.ascontiguousarray(zcs[0][:, 512:])
        m["zc1"] = zcs[1]
        m["blob0"] = np.ascontiguousarray(
            np.concatenate([zl0, ident, ibig, zcs[0][:, :512]], axis=1))
        m["blob1"] = zl1
        m["aug"] = aug.astype(bf)
        in_maps.append(m)
    return in_maps


def kernel(z0: np.ndarray, z1: np.ndarray) -> np.ndarray:
    from concourse.bass_utils import run_bass_kernel_spmd

    if "nc" not in _cache:
        _cache["nc"] = _build()
    nc = _cache["nc"]

    in_maps = _prep_inputs(z0, z1)
    res = run_bass_kernel_spmd(nc, in_maps, core_ids=list(range(N_CORES)))

    rowsums = np.zeros((2, B), np.float64)   # [view, global row]
    for c in range(N_CORES):
        out = res.results[c]
        escale = float(np.exp(-CEXP))
        acc = out["accs"].astype(np.float64) * escale      # [128, 2]
        rowsums[1, c * R + 128:c * R + 256] += acc[:, 0] + acc[:, 1]
        if c < 4:
            k4rows = [np.arange((c + 4) * R + 128 * t,
                                (c + 4) * R + 128 * t + 128) for t in (0, 1)]
        else:
            k4rows = [np.arange((c - 4) * R + 128 * (1 - t),
                                (c - 4) * R + 128 * (1 - t) + 128)
                      for t in (0, 1)]
        for name, v, t, c0, lo, hi in DUMPS:
            slab = out[name].astype(np.float64) * escale    # [128, hi-lo]
            # own rows: sum along columns (u11's rows come from accums)
            if name != "d11a":
                r0 = c * R + t * 128
                rowsums[v, r0:r0 + 128] += slab.sum(axis=1)
            # mirrored rows, blocks k=1..3: sum along partitions
            mlo, mhi = max(lo, 256), min(hi, 1024)
            if mlo < mhi:
                cs = slab[:, mlo - lo:mhi - lo].sum(axis=0)
                grows = (np.arange(mlo, mhi) + c * R) % B
                rowsums[v, grows] += cs
            # mirrored rows, k4 half-block [1024:1152)
            if hi == GC:
                cs = slab[:, 1024 - lo:GC - lo].sum(axis=0)
                rowsums[v, k4rows[t]] += cs

    z0f = z0.astype(np.float64)
    z1f = z1.astype(np.float64)
    align_loss = np.sqrt(((z0f - z1f) ** 2).sum(-1)).mean()
    lme = np.log(rowsums) - LOG_NM1             # [2, B]
    entropy_loss = lme.mean()
    return np.float32(align_loss - entropy_loss)


# revision 52
# speedup vs baseline: 1.0494x; 1.0011x over previous
"""Trainium2 Bass kernel for LpAlignEntropyLoss (B=2048, D=128, 2 views).

loss = mean_i ||z0_i - z1_i + eps||  -  0.5 * sum_v mean_i [ logsumexp_{j!=i}(-||zv_i - zv_j + eps||) - log(B-1) ]

Symmetric block scheme (8 NeuronCores, 256 rows/core):
  The BxB distance matrix is symmetric, so core c only computes blocks
  (c, c..c+4): gathered columns are the 1280 cyclically-next rows. Row
  sums come from the fused ACT accum; the mirrored contributions for
  blocks k=1..3 are column sums of the exp tiles, which are DMA'd out
  and reduced on the host. Block k=4 is computed by both endpoints
  (row-sums only), keeping the SPMD program uniform.

  dist^2[i,j] = n_i + n_j - 2 z_i.z_j, assembled fully in PSUM:
  - PE: psum = (-2 z_i).z_j (bf16 lhsT, host-prescaled) + [1;n_i]x[n_j;1]
    (K=2 aug matmul) + BIG*I (identity matmul, masks the diagonal).
  - DVE: sqrt via the fp32 bit trick -- psum bitcast to int32,
    dist_bits = 0.5*i + MAGIC (one tensor_scalar mult+add).  MAGIC is
    tuned so the logsumexp bias cancels (validated ~3e-7 rel).
  - ACT: Exp(-dist) on dist bitcast to f32, fused accum_out row-sum.
    Only the exp table is ever loaded (preloaded at t=0 via a dummy).
  Host finishes the tail: align term, mirror column sums, log, means.
"""
import numpy as np
import ml_dtypes
from contextlib import ExitStack

B = 2048
D = 128
N_CORES = 8
R = B // N_CORES          # 256 rows per core
G = 1280                  # zc layout columns (k4 halves for both chunks)
GC = 1152                 # per-chunk compute span (k4 block halved)
MAGIC = 532626640.0       # sqrt bit-trick offset, tuned on the data model
CEXP = 16.5               # exp bias: dumps hold exp(CEXP - d) in fp8 e4m3
BIG = float(2 ** 20)
LOG_NM1 = float(np.log(B - 1))

# dump slabs DMA'd out; host derives both row sums and mirror column sums:
#   name -> (view, chunk, piece_c0, slice_lo, slice_hi)  (chunk-space cols)
DUMPS = [
    ("d00a", 0, 0, 0, 0, 512),
    ("d00b", 0, 0, 512, 512, GC),
    ("d01", 0, 1, 0, 0, GC),
    ("d10", 1, 0, 0, 0, GC),
    ("d11a", 1, 1, 256, 256, GC),
]

_cache: dict = {}


def _build():
    import concourse.tile as tile
    from concourse import bacc, mybir

    f32 = mybir.dt.float32
    bf16 = mybir.dt.bfloat16
    f8 = mybir.dt.float8e4
    i32 = mybir.dt.int32
    AF = mybir.ActivationFunctionType
    ALU = mybir.AluOpType

    nc = bacc.Bacc("TRN2", target_bir_lowering=False, debug=False,
                   num_devices=N_CORES)

    zc0b_d = nc.dram_tensor("zc0b", [D, G - 512], bf16, kind="ExternalInput").ap()
    zc1_d = nc.dram_tensor("zc1", [D, G], bf16, kind="ExternalInput").ap()
    # blob0 cols: zl0(256) ident(128) ibig(128); blob1: zl1(256)
    blob0_d = nc.dram_tensor("blob0", [128, 1024], bf16, kind="ExternalInput").ap()
    blob1_d = nc.dram_tensor("blob1", [128, 256], bf16, kind="ExternalInput").ap()
    # aug cols per view: [2,256] lhsT region (1; n_own), [2,G] rhs (n_gath; 1)
    aug_d = nc.dram_tensor("aug", [2, 2 * (256 + G)], bf16,
                           kind="ExternalInput").ap()
    accs_d = nc.dram_tensor("accs", [128, 2], f32, kind="ExternalOutput").ap()
    dump_d = {name: nc.dram_tensor(name, [128, hi - lo], f8,
                                   kind="ExternalOutput").ap()
              for name, v, t, c0, lo, hi in DUMPS}

    with tile.TileContext(nc) as tc, ExitStack() as ctx:
        consts = ctx.enter_context(tc.tile_pool(name="consts", bufs=1))
        psum = ctx.enter_context(tc.tile_pool(name="psum", bufs=4, space="PSUM"))
        distp = ctx.enter_context(tc.tile_pool(name="distp", bufs=3))
        dumpp = ctx.enter_context(tc.tile_pool(name="dumpp", bufs=6))

        # --- ACT exp-table preload (no data deps) ---
        dumm = consts.tile([128, 1], f32, tag="dumm")
        nc.vector.memset(dumm[:], 0.0)
        cexp = consts.tile([128, 1], f32, tag="cexp")
        nc.vector.memset(cexp[:], CEXP)
        dumo = consts.tile([128, 1], bf16, tag="dumo")
        nc.scalar.activation(dumo[:], dumm[:], AF.Exp, scale=-1.0)

        # --- PE p-state warm-up: small matmuls from t~0.3us keep PE busy
        # until real work arrives, so real matmuls run at 2.4GHz ---
        dumw = consts.tile([128, 128], bf16, tag="dumw")
        nc.gpsimd.memset(dumw[:], 0.0)
        dumP = psum.tile([128, 1024], f32, tag="P")
        for _ in range(26):
            nc.tensor.matmul(dumP[:, 0:128], dumw[:], dumw[:],
                             start=True, stop=True)

        # --- inputs ---
        sb_zc0 = consts.tile([D, G], bf16, tag="zc0")
        sb_zc1 = consts.tile([D, G], bf16, tag="zc1")
        sb_zc = [sb_zc0, sb_zc1]
        blob0 = consts.tile([128, 1024], bf16, tag="blob0")
        blob1 = consts.tile([128, 256], bf16, tag="blob1")
        aug = consts.tile([2, 2 * (256 + G)], bf16, tag="aug")
        nc.sync.dma_start(blob0[:], blob0_d)
        nc.sync.dma_start(aug[:], aug_d)
        nc.sync.dma_start(sb_zc[0][:, 512:G], zc0b_d)
        nc.sync.dma_start(sb_zc[1][:], zc1_d)
        nc.sync.dma_start(blob1[:], blob1_d)

        zl = [blob0[:, 0:256], blob1[:, 0:256]]
        ident = blob0[:, 256:384]
        ibig = blob0[:, 384:512]
        augl = [aug[:, 0:256], aug[:, 256 + G:512 + G]]
        augr = [aug[:, 256:256 + G], aug[:, 512 + G:512 + 2 * G]]

        accs = consts.tile([128, 2], f32, tag="accs")
        dists = {}
        dumps = {}

        def mm_trick(v, t, c0, c1, dist, act_trick=False):
            """matmuls + sqrt-trick for gathered cols [c0,c1) into dist."""
            ncols = c1 - c0
            P = psum.tile([128, ncols], f32, tag="P")
            lhsT = zl[v][:, t * 128:(t + 1) * 128]
            auglT = augl[v][:, t * 128:(t + 1) * 128]
            dlo, dhi = t * 128, (t + 1) * 128
            has_diag = c0 <= dlo and dhi <= c1
            cuts = list(range(c0, min(c1, 1024), 512)) + (
                [1024] if c1 > 1024 else [])
            for s0 in cuts:
                s1 = min(s0 + 512, c1) if s0 < 1024 else c1
                s1 = min(s1, 1024) if s0 < 1024 else s1
                sl = slice(s0 - c0, s1 - c0)
                # chunk-space [1024:1152) maps to zc cols [1024+128t, ...)
                z0c, z1c = (s0, s1) if s0 < 1024 else (s0 + 128 * t,
                                                       s1 + 128 * t)
                rhs = (blob0[:, 512 + z0c:512 + z1c] if v == 0 and z1c <= 512
                       else sb_zc[v][:, z0c:z1c])
                nc.tensor.matmul(P[:, sl], lhsT, rhs,
                                 start=True, stop=False)
                last = not (has_diag and s0 <= dlo < s1)
                nc.tensor.matmul(P[:, sl], auglT, augr[v][:, z0c:z1c],
                                 start=False, stop=last)
            if has_diag:
                nc.tensor.matmul(P[:, dlo - c0:dhi - c0], ident, ibig,
                                 start=False, stop=True)
            if act_trick:
                # ACT is idle before its exp chain: same 0.5*i + MAGIC math
                # via Copy (float bias allowed), int32-convert on write
                nc.scalar.activation(dist[:, c0:c1].bitcast(i32),
                                     P[:].bitcast(i32), AF.Copy,
                                     bias=MAGIC, scale=0.5)
            else:
                nc.vector.tensor_scalar(dist[:, c0:c1].bitcast(i32),
                                        P[:].bitcast(i32), 0.5, MAGIC,
                                        ALU.mult, ALU.add)

        def exp_acc(v, t, c0, c1, acc_idx=None):
            """exp over dist cols [c0,c1); host sums the dumped slab,
            except the last unit which uses the fused accum row-sum."""
            dmp = dumpp.tile([128, c1 - c0], f8, tag="dump")
            acc = None if acc_idx is None else accs[:, acc_idx:acc_idx + 1]
            nc.scalar.activation(dmp[:], dists[(v, t)][:, c0:c1], AF.Exp,
                                 scale=-1.0, bias=cexp[:], accum_out=acc)
            dumps[(v, t, c0)] = dmp

        def dump_out(name, v, t, c0, lo, hi):
            dmp = dumps[(v, t, c0)]
            nc.sync.dma_start(dump_d[name], dmp[:, lo - c0:hi - c0])

        for v, t in [(0, 0), (0, 1), (1, 0), (1, 1)]:
            dtile = distp.tile([128, GC], f32, tag=f"dist{v}{t}")
            dists[(v, t)] = dtile

        # u00 in 2 pieces for an early ACT start; u11 split so its mirror
        # slab (cols 256..1024) ships before the final k4 piece finishes.
        mm_trick(0, 0, 0, 512, dists[(0, 0)], act_trick=True)
        exp_acc(0, 0, 0, 512)
        mm_trick(0, 0, 512, GC, dists[(0, 0)])
        exp_acc(0, 0, 512, GC)
        dump_out(*DUMPS[0])
        mm_trick(0, 1, 0, 768, dists[(0, 1)])
        mm_trick(0, 1, 768, GC, dists[(0, 1)])
        exp_acc(0, 1, 0, GC)
        dump_out(*DUMPS[1])
        mm_trick(1, 0, 0, 768, dists[(1, 0)])
        mm_trick(1, 0, 768, GC, dists[(1, 0)])
        exp_acc(1, 0, 0, GC)
        dump_out(*DUMPS[2])
        mm_trick(1, 1, 256, GC, dists[(1, 1)])
        dump_out(*DUMPS[3])
        exp_acc(1, 1, 256, GC, 0)
        dump_out(*DUMPS[4])
        mm_trick(1, 1, 0, 256, dists[(1, 1)])
        exp_acc(1, 1, 0, 256, 1)
        nc.sync.dma_start(accs_d, accs[:])

    nc.compile()
    return nc


def _prep_inputs(z0: np.ndarray, z1: np.ndarray):
    """Per-core input maps: gathered columns are the cyclically-next 1280."""
    bf = ml_dtypes.bfloat16
    zs = [np.ascontiguousarray(z0, np.float32), np.ascontiguousarray(z1, np.float32)]
    norms = [(z.astype(np.float64) ** 2).sum(-1) for z in zs]  # [B]
    eye = np.eye(128, dtype=np.float32)
    ident = eye.astype(bf)
    ibig = (BIG * eye).astype(bf)
    in_maps = []
    for c in range(N_CORES):
        base = (np.arange(1024) + c * R) % B
        if c < 4:
            k4t = [np.arange((c + 4) * R + 128 * t,
                             (c + 4) * R + 128 * t + 128) for t in (0, 1)]
        else:
            k4t = [np.arange((c - 4) * R + 128 * (1 - t),
                             (c - 4) * R + 128 * (1 - t) + 128) for t in (0, 1)]
        gcols = np.concatenate([base, k4t[0], k4t[1]]) % B
        m = {}
        aug = np.zeros((2, 2 * (256 + G)), np.float32)
        zcs = []
        for v in (0, 1):
            zc = np.ascontiguousarray(zs[v][gcols].T).astype(bf)   # [D, G]
            zcs.append(zc)
            ng = norms[v][gcols].astype(np.float32)
            o = v * (256 + G)
            aug[0, o:o + 256] = 1.0
            aug[1, o:o + 256] = ng[:256]
            aug[0, o + 256:o + 256 + G] = ng
            aug[1, o + 256:o + 256 + G] = 1.0
        zl0 = (-2.0 * zcs[0][:, :256].astype(np.float32)).astype(bf)
        zl1 = (-2.0 * zcs[1][:, :256].astype(np.float32)).astype(bf)
        m["zc0b"] = np.ascontiguousarray(zcs[0][:, 512:])
        m["zc1"] = zcs[1]
        m["blob0"] = np.ascontiguousarray(
            np.concatenate([zl0, ident, ibig, zcs[0][:, :512]], axis=1))
        m["blob1"] = zl1
        m["aug"] = aug.astype(bf)
        in_maps.append(m)
    return in_maps


def kernel(z0: np.ndarray, z1: np.ndarray) -> np.ndarray:
    from concourse.bass_utils import run_bass_kernel_spmd

    if "nc" not in _cache:
        _cache["nc"] = _build()
    nc = _cache["nc"]

    in_maps = _prep_inputs(z0, z1)
    res = run_bass_kernel_spmd(nc, in_maps, core_ids=list(range(N_CORES)))

    rowsums = np.zeros((2, B), np.float64)   # [view, global row]
    for c in range(N_CORES):
        out = res.results[c]
        escale = float(np.exp(-CEXP))
        acc = out["accs"].astype(np.float64) * escale      # [128, 2]
        rowsums[1, c * R + 128:c * R + 256] += acc[:, 0] + acc[:, 1]
        if c < 4:
            k4rows = [np.arange((c + 4) * R + 128 * t,
                                (c + 4) * R + 128 * t + 128) for t in (0, 1)]
        else:
            k4rows = [np.arange((c - 4) * R + 128 * (1 - t),
                                (c - 4) * R + 128 * (1 - t) + 128)
                      for t in (0, 1)]
        for name, v, t, c0, lo, hi in DUMPS:
            slab = out[name].astype(np.float64) * escale    # [128, hi-lo]
            # own rows: sum along columns (u11's rows come from accums)
            if name != "d11a":
                r0 = c * R + t * 128
                rowsums[v, r0:r0 + 128] += slab.sum(axis=1)
            # mirrored rows, blocks k=1..3: sum along partitions
            mlo, mhi = max(lo, 256), min(hi, 1024)
            if mlo < mhi:
                cs = slab[:, mlo - lo:mhi - lo].sum(axis=0)
                grows = (np.arange(mlo, mhi) + c * R) % B
                rowsums[v, grows] += cs
            # mirrored rows, k4 half-block [1024:1152)
            if hi == GC:
                cs = slab[:, 1024 - lo:GC - lo].sum(axis=0)
                rowsums[v, k4rows[t]] += cs

    z0f = z0.astype(np.float64)
    z1f = z1.astype(np.float64)
    align_loss = np.sqrt(((z0f - z1f) ** 2).sum(-1)).mean()
    lme = np.log(rowsums) - LOG_NM1             # [2, B]
    entropy_loss = lme.mean()
    return np.float32(align_loss - entropy_loss)


# revision 53
# speedup vs baseline: 1.0569x; 1.0072x over previous
"""Trainium2 Bass kernel for LpAlignEntropyLoss (B=2048, D=128, 2 views).

loss = mean_i ||z0_i - z1_i + eps||  -  0.5 * sum_v mean_i [ logsumexp_{j!=i}(-||zv_i - zv_j + eps||) - log(B-1) ]

Symmetric block scheme (8 NeuronCores, 256 rows/core):
  The BxB distance matrix is symmetric, so core c only computes blocks
  (c, c..c+4): gathered columns are the 1280 cyclically-next rows. Row
  sums come from the fused ACT accum; the mirrored contributions for
  blocks k=1..3 are column sums of the exp tiles, which are DMA'd out
  and reduced on the host. Block k=4 is computed by both endpoints
  (row-sums only), keeping the SPMD program uniform.

  dist^2[i,j] = n_i + n_j - 2 z_i.z_j, assembled fully in PSUM:
  - PE: psum = (-2 z_i).z_j (bf16 lhsT, host-prescaled) + [1;n_i]x[n_j;1]
    (K=2 aug matmul) + BIG*I (identity matmul, masks the diagonal).
  - DVE: sqrt via the fp32 bit trick -- psum bitcast to int32,
    dist_bits = 0.5*i + MAGIC (one tensor_scalar mult+add).  MAGIC is
    tuned so the logsumexp bias cancels (validated ~3e-7 rel).
  - ACT: Exp(-dist) on dist bitcast to f32, fused accum_out row-sum.
    Only the exp table is ever loaded (preloaded at t=0 via a dummy).
  Host finishes the tail: align term, mirror column sums, log, means.
"""
import numpy as np
import ml_dtypes
from contextlib import ExitStack

B = 2048
D = 128
N_CORES = 8
R = B // N_CORES          # 256 rows per core
G = 1280                  # zc layout columns (k4 halves for both chunks)
GC = 1152                 # per-chunk compute span (k4 block halved)
MAGIC = 532626640.0       # sqrt bit-trick offset, tuned on the data model
CEXP = 16.5               # exp bias: dumps hold exp(CEXP - d) in fp8 e4m3
BIG = float(2 ** 20)
LOG_NM1 = float(np.log(B - 1))

# dump slabs DMA'd out; host derives both row sums and mirror column sums:
#   name -> (view, chunk, piece_c0, slice_lo, slice_hi)  (chunk-space cols)
DUMPS = [
    ("d00a", 0, 0, 0, 0, 512),
    ("d00b", 0, 0, 512, 512, GC),
    ("d01", 0, 1, 0, 0, GC),
    ("d10", 1, 0, 0, 0, GC),
    ("d11a", 1, 1, 256, 256, GC),
]

_cache: dict = {}


def _build():
    import concourse.tile as tile
    from concourse import bacc, mybir

    f32 = mybir.dt.float32
    bf16 = mybir.dt.bfloat16
    f8 = mybir.dt.float8e4
    i32 = mybir.dt.int32
    AF = mybir.ActivationFunctionType
    ALU = mybir.AluOpType

    nc = bacc.Bacc("TRN2", target_bir_lowering=False, debug=False,
                   num_devices=N_CORES)

    zc0b_d = nc.dram_tensor("zc0b", [D, G - 512], f8, kind="ExternalInput").ap()
    zc1_d = nc.dram_tensor("zc1", [D, G], f8, kind="ExternalInput").ap()
    # blob0 cols: zl0(256) ident(128) ibig(128); blob1: zl1(256)
    blob0_d = nc.dram_tensor("blob0", [128, 1024], bf16, kind="ExternalInput").ap()
    blob1_d = nc.dram_tensor("blob1", [128, 256], bf16, kind="ExternalInput").ap()
    # aug cols per view: [2,256] lhsT region (1; n_own), [2,G] rhs (n_gath; 1)
    aug_d = nc.dram_tensor("aug", [2, 2 * (256 + G)], bf16,
                           kind="ExternalInput").ap()
    accs_d = nc.dram_tensor("accs", [128, 2], f32, kind="ExternalOutput").ap()
    dump_d = {name: nc.dram_tensor(name, [128, hi - lo], f8,
                                   kind="ExternalOutput").ap()
              for name, v, t, c0, lo, hi in DUMPS}

    with tile.TileContext(nc) as tc, ExitStack() as ctx:
        consts = ctx.enter_context(tc.tile_pool(name="consts", bufs=1))
        psum = ctx.enter_context(tc.tile_pool(name="psum", bufs=4, space="PSUM"))
        distp = ctx.enter_context(tc.tile_pool(name="distp", bufs=3))
        dumpp = ctx.enter_context(tc.tile_pool(name="dumpp", bufs=6))

        # --- ACT exp-table preload (no data deps) ---
        dumm = consts.tile([128, 1], f32, tag="dumm")
        nc.vector.memset(dumm[:], 0.0)
        cexp = consts.tile([128, 1], f32, tag="cexp")
        nc.vector.memset(cexp[:], CEXP)
        dumo = consts.tile([128, 1], bf16, tag="dumo")
        nc.scalar.activation(dumo[:], dumm[:], AF.Exp, scale=-1.0)

        # --- PE p-state warm-up: small matmuls from t~0.3us keep PE busy
        # until real work arrives, so real matmuls run at 2.4GHz ---
        dumw = consts.tile([128, 128], bf16, tag="dumw")
        nc.gpsimd.memset(dumw[:], 0.0)
        dumP = psum.tile([128, 1024], f32, tag="P")
        for _ in range(26):
            nc.tensor.matmul(dumP[:, 0:128], dumw[:], dumw[:],
                             start=True, stop=True)

        # --- inputs ---
        sb_zc0 = consts.tile([D, G], f8, tag="zc0")
        sb_zc1 = consts.tile([D, G], f8, tag="zc1")
        sb_zc = [sb_zc0, sb_zc1]
        blob0 = consts.tile([128, 1024], bf16, tag="blob0")
        blob1 = consts.tile([128, 256], bf16, tag="blob1")
        aug = consts.tile([2, 2 * (256 + G)], bf16, tag="aug")
        nc.sync.dma_start(blob0[:], blob0_d)
        nc.sync.dma_start(aug[:], aug_d)
        nc.sync.dma_start(sb_zc[0][:, 512:G], zc0b_d)
        nc.sync.dma_start(sb_zc[1][:], zc1_d)
        nc.sync.dma_start(blob1[:], blob1_d)

        zl = [blob0[:, 0:256], blob1[:, 0:256]]
        ident = blob0[:, 256:384]
        ibig = blob0[:, 384:512]
        augl = [aug[:, 0:256], aug[:, 256 + G:512 + G]]
        augr = [aug[:, 256:256 + G], aug[:, 512 + G:512 + 2 * G]]

        accs = consts.tile([128, 2], f32, tag="accs")
        dists = {}
        dumps = {}

        def mm_trick(v, t, c0, c1, dist, act_trick=False):
            """matmuls + sqrt-trick for gathered cols [c0,c1) into dist."""
            ncols = c1 - c0
            P = psum.tile([128, ncols], f32, tag="P")
            lhsT = zl[v][:, t * 128:(t + 1) * 128]
            auglT = augl[v][:, t * 128:(t + 1) * 128]
            dlo, dhi = t * 128, (t + 1) * 128
            has_diag = c0 <= dlo and dhi <= c1
            cuts = list(range(c0, min(c1, 1024), 512)) + (
                [1024] if c1 > 1024 else [])
            for s0 in cuts:
                s1 = min(s0 + 512, c1) if s0 < 1024 else c1
                s1 = min(s1, 1024) if s0 < 1024 else s1
                sl = slice(s0 - c0, s1 - c0)
                # chunk-space [1024:1152) maps to zc cols [1024+128t, ...)
                z0c, z1c = (s0, s1) if s0 < 1024 else (s0 + 128 * t,
                                                       s1 + 128 * t)
                rhs = (blob0[:, 512 + z0c:512 + z1c] if v == 0 and z1c <= 512
                       else sb_zc[v][:, z0c:z1c])
                nc.tensor.matmul(P[:, sl], lhsT, rhs,
                                 start=True, stop=False)
                last = not (has_diag and s0 <= dlo < s1)
                nc.tensor.matmul(P[:, sl], auglT, augr[v][:, z0c:z1c],
                                 start=False, stop=last)
            if has_diag:
                nc.tensor.matmul(P[:, dlo - c0:dhi - c0], ident, ibig,
                                 start=False, stop=True)
            if act_trick:
                # ACT is idle before its exp chain: same 0.5*i + MAGIC math
                # via Copy (float bias allowed), int32-convert on write
                nc.scalar.activation(dist[:, c0:c1].bitcast(i32),
                                     P[:].bitcast(i32), AF.Copy,
                                     bias=MAGIC, scale=0.5)
            else:
                nc.vector.tensor_scalar(dist[:, c0:c1].bitcast(i32),
                                        P[:].bitcast(i32), 0.5, MAGIC,
                                        ALU.mult, ALU.add)

        def exp_acc(v, t, c0, c1, acc_idx=None):
            """exp over dist cols [c0,c1); host sums the dumped slab,
            except the last unit which uses the fused accum row-sum."""
            dmp = dumpp.tile([128, c1 - c0], f8, tag="dump")
            acc = None if acc_idx is None else accs[:, acc_idx:acc_idx + 1]
            nc.scalar.activation(dmp[:], dists[(v, t)][:, c0:c1], AF.Exp,
                                 scale=-1.0, bias=cexp[:], accum_out=acc)
            dumps[(v, t, c0)] = dmp

        def dump_out(name, v, t, c0, lo, hi):
            dmp = dumps[(v, t, c0)]
            nc.sync.dma_start(dump_d[name], dmp[:, lo - c0:hi - c0])

        for v, t in [(0, 0), (0, 1), (1, 0), (1, 1)]:
            dtile = distp.tile([128, GC], f32, tag=f"dist{v}{t}")
            dists[(v, t)] = dtile

        # u00 in 2 pieces for an early ACT start; u11 split so its mirror
        # slab (cols 256..1024) ships before the final k4 piece finishes.
        mm_trick(0, 0, 0, 512, dists[(0, 0)], act_trick=True)
        exp_acc(0, 0, 0, 512)
        mm_trick(0, 0, 512, GC, dists[(0, 0)])
        exp_acc(0, 0, 512, GC)
        dump_out(*DUMPS[0])
        mm_trick(0, 1, 0, 768, dists[(0, 1)])
        mm_trick(0, 1, 768, GC, dists[(0, 1)])
        exp_acc(0, 1, 0, GC)
        dump_out(*DUMPS[1])
        mm_trick(1, 0, 0, 768, dists[(1, 0)])
        mm_trick(1, 0, 768, GC, dists[(1, 0)])
        exp_acc(1, 0, 0, GC)
        dump_out(*DUMPS[2])
        mm_trick(1, 1, 256, GC, dists[(1, 1)])
        dump_out(*DUMPS[3])
        exp_acc(1, 1, 256, GC, 0)
        dump_out(*DUMPS[4])
        mm_trick(1, 1, 0, 256, dists[(1, 1)])
        exp_acc(1, 1, 0, 256, 1)
        nc.sync.dma_start(accs_d, accs[:])

    nc.compile()
    return nc


def _prep_inputs(z0: np.ndarray, z1: np.ndarray):
    """Per-core input maps: gathered columns are the cyclically-next 1280."""
    bf = ml_dtypes.bfloat16
    zs = [np.ascontiguousarray(z0, np.float32), np.ascontiguousarray(z1, np.float32)]
    norms = [(z.astype(np.float64) ** 2).sum(-1) for z in zs]  # [B]
    eye = np.eye(128, dtype=np.float32)
    ident = eye.astype(bf)
    ibig = (BIG * eye).astype(bf)
    in_maps = []
    for c in range(N_CORES):
        base = (np.arange(1024) + c * R) % B
        if c < 4:
            k4t = [np.arange((c + 4) * R + 128 * t,
                             (c + 4) * R + 128 * t + 128) for t in (0, 1)]
        else:
            k4t = [np.arange((c - 4) * R + 128 * (1 - t),
                             (c - 4) * R + 128 * (1 - t) + 128) for t in (0, 1)]
        gcols = np.concatenate([base, k4t[0], k4t[1]]) % B
        m = {}
        aug = np.zeros((2, 2 * (256 + G)), np.float32)
        zcs = []
        for v in (0, 1):
            zc = np.ascontiguousarray(zs[v][gcols].T).astype(bf)   # [D, G]
            zcs.append(zc)
            ng = norms[v][gcols].astype(np.float32)
            o = v * (256 + G)
            aug[0, o:o + 256] = 1.0
            aug[1, o:o + 256] = ng[:256]
            aug[0, o + 256:o + 256 + G] = ng
            aug[1, o + 256:o + 256 + G] = 1.0
        zl0 = (-2.0 * zcs[0][:, :256].astype(np.float32)).astype(bf)
        zl1 = (-2.0 * zcs[1][:, :256].astype(np.float32)).astype(bf)
        f8i = ml_dtypes.float8_e4m3
        m["zc0b"] = np.ascontiguousarray(zcs[0][:, 512:]).astype(f8i)
        m["zc1"] = zcs[1].astype(f8i)
        m["blob0"] = np.ascontiguousarray(
            np.concatenate([zl0, ident, ibig, zcs[0][:, :512]], axis=1))
        m["blob1"] = zl1
        m["aug"] = aug.astype(bf)
        in_maps.append(m)
    return in_maps


def kernel(z0: np.ndarray, z1: np.ndarray) -> np.ndarray:
    from concourse.bass_utils import run_bass_kernel_spmd

    if "nc" not in _cache:
        _cache["nc"] = _build()
    nc = _cache["nc"]

    in_maps = _prep_inputs(z0, z1)
    res = run_bass_kernel_spmd(nc, in_maps, core_ids=list(range(N_CORES)))

    rowsums = np.zeros((2, B), np.float64)   # [view, global row]
    for c in range(N_CORES):
        out = res.results[c]
        escale = float(np.exp(-CEXP))
        acc = out["accs"].astype(np.float64) * escale      # [128, 2]
        rowsums[1, c * R + 128:c * R + 256] += acc[:, 0] + acc[:, 1]
        if c < 4:
            k4rows = [np.arange((c + 4) * R + 128 * t,
                                (c + 4) * R + 128 * t + 128) for t in (0, 1)]
        else:
            k4rows = [np.arange((c - 4) * R + 128 * (1 - t),
                                (c - 4) * R + 128 * (1 - t) + 128)
                      for t in (0, 1)]
        for name, v, t, c0, lo, hi in DUMPS:
            slab = out[name].astype(np.float64) * escale    # [128, hi-lo]
            # own rows: sum along columns (u11's rows come from accums)
            if name != "d11a":
                r0 = c * R + t * 128
                rowsums[v, r0:r0 + 128] += slab.sum(axis=1)
            # mirrored rows, blocks k=1..3: sum along partitions
            mlo, mhi = max(lo, 256), min(hi, 1024)
            if mlo < mhi:
                cs = slab[:, mlo - lo:mhi - lo].sum(axis=0)
                grows = (np.arange(mlo, mhi) + c * R) % B
                rowsums[v, grows] += cs
            # mirrored rows, k4 half-block [1024:1152)
            if hi == GC:
                cs = slab[:, 1024 - lo:GC - lo].sum(axis=0)
                rowsums[v, k4rows[t]] += cs

    z0f = z0.astype(np.float64)
    z1f = z1.astype(np.float64)
    align_loss = np.sqrt(((z0f - z1f) ** 2).sum(-1)).mean()
    lme = np.log(rowsums) - LOG_NM1             # [2, B]
    entropy_loss = lme.mean()
    return np.float32(align_loss - entropy_loss)
